# revision 1
# baseline (speedup 1.0000x reference)
"""Trainium2 Bass kernel for nn_LossFunction_2740189135094 (AAM-softmax +
score-normalized angle-proto speaker loss).

Contract: kernel(**inputs) takes FULL unsharded inputs (as produced by the
reference setup_inputs) and returns the full output: a (2,) float32 array
[nlossS + nlossP, prec1].

Strategy (8 NeuronCores, no collectives needed — tiny partial outputs are
merged on host):
  Phase A (class-sharded): cosine = l2norm(x) @ l2norm(weight).T computed in
    fp8-e4m3 DoubleRow on the PE (2x rate); each core owns 752 of the
    (padded-to-6016) 5994 classes and produces, for all 4096 rows: partial
    sum(exp(30*cos)) via the ACT engine's fused accum_out, and partial
    row-max via the DVE.
  Phase B (anchor-sharded): D = Xp @ Xa.T over the 2048 paired embeddings;
    each core owns 256 anchor columns and ships the raw bf16 D tiles; the
    exp/row/column sums happen on host.
  Host: l2-normalization / transposes / fp8 quantization of inputs, the
    label-gathered target cosines (computed from the same fp8-quantized
    operands the device sees), and the final logs and means in float64.

The top-k cohort statistics in the reference are multiplied by w2/b2; for the
actual inputs w2 == b2 == 0, so csm is an affine function of out_dot and p2's
matrix is exactly p1's transpose. If w2/b2 were nonzero we fall back to an
exact numpy implementation.
"""

import math
import sys

import numpy as np

for _p in ("/opt/trn_rl_repo", "/opt/pypackages"):
    if _p not in sys.path:
        sys.path.insert(0, _p)

import ml_dtypes  # noqa: E402

NOUT = 512
NCLS = 5994
B = 2048
R = 4096  # 2 * B rows
NCORES = 8
CSH = 752  # padded class shard: 8 * 752 = 6016 >= 5994
NPAD = NCORES * CSH - NCLS  # 22 zero-padded classes on the last core
ASH = B // NCORES  # 256 anchors per core
MARGIN = 0.2
SCALE = 30.0

_COS_M = math.cos(MARGIN)
_SIN_M = math.sin(MARGIN)
_TH = math.cos(math.pi - MARGIN)
_MM = math.sin(math.pi - MARGIN) * MARGIN

_cache: dict = {}

# Results of the last device run (for the test harness to inspect timing).
last_results = None


def _hsig(v):
    return np.clip((v + 3.0) / 6.0, 0.0, 1.0)


def _build_program():
    import concourse.mybir as mybir
    import concourse.tile as tile
    from concourse import bacc
    from contextlib import ExitStack

    bf16 = mybir.dt.bfloat16
    f8 = mybir.dt.float8e4
    f32 = mybir.dt.float32
    DR = mybir.MatmulPerfMode.DoubleRow

    nc = bacc.Bacc(
        "TRN2", target_bir_lowering=False, debug=False, num_devices=NCORES
    )
    xpt = nc.dram_tensor("xpt", [NOUT, B], f8, kind="ExternalInput").ap()
    xat = nc.dram_tensor("xat", [NOUT, B], f8, kind="ExternalInput").ap()
    xash = nc.dram_tensor("xash", [NOUT, ASH], f8, kind="ExternalInput").ap()
    wnt = nc.dram_tensor("wnt", [NOUT, CSH], f8, kind="ExternalInput").ap()
    o_se = nc.dram_tensor("o_se", [128, 32], f32, kind="ExternalOutput").ap()
    o_mx = nc.dram_tensor("o_mx", [128, 32], f32, kind="ExternalOutput").ap()
    # raw D = Xp @ Xa_shard.T tiles; exp/log-sum-exp done on host
    o_d = nc.dram_tensor("o_d", [16, 128, ASH], bf16, kind="ExternalOutput").ap()

    EXP = mybir.ActivationFunctionType.Exp
    AX = mybir.AxisListType.X

    with tile.TileContext(nc) as tc, ExitStack() as ctx:
        consts = ctx.enter_context(tc.tile_pool(name="consts", bufs=1))
        psums = ctx.enter_context(tc.tile_pool(name="psums", bufs=3, space="PSUM"))
        psumsB = ctx.enter_context(tc.tile_pool(name="psumsB", bufs=2, space="PSUM"))
        scratch = ctx.enter_context(tc.tile_pool(name="scratch", bufs=3))

        # PE warm-up fodder: a few matmuls on scratch data keep the HAM
        # activity window busy while the real inputs stream in, so the PE
        # clock is at 2.4 GHz (not 1.2) when the first real matmul issues.
        # Initialized on the otherwise-idle GpSimd so the warm-up matmuls
        # don't wait on DVE/ACT.
        warm = consts.tile([128, 512], bf16)
        nc.gpsimd.memset(warm, 0.0)

        # Load everything to SBUF once, k-dim split into 4 partition chunks.
        # wnt + the first xpt column chunk gate the first real matmul, so
        # they go first and xpt/xat are split into column chunks.
        s_wnt = consts.tile([128, 2, 2, CSH], f8)
        nc.sync.dma_start(
            out=s_wnt, in_=wnt.rearrange("(c r p) n -> p c r n", p=128, r=2)
        )
        xpt_r = xpt.rearrange("(c r p) n -> p c r n", p=128, r=2)
        xat_r = xat.rearrange("(c r p) n -> p c r n", p=128, r=2)
        s_xpt = consts.tile([128, 2, 2, B], f8)
        s_xat = consts.tile([128, 2, 2, B], f8)
        for q in range(4):
            nc.sync.dma_start(
                out=s_xpt[:, :, :, q * 512 : (q + 1) * 512],
                in_=xpt_r[:, :, :, q * 512 : (q + 1) * 512],
            )
        s_xash = consts.tile([128, 2, 2, ASH], f8)
        nc.sync.dma_start(
            out=s_xash, in_=xash.rearrange("(c r p) n -> p c r n", p=128, r=2)
        )
        for q in range(4):
            nc.sync.dma_start(
                out=s_xat[:, :, :, q * 512 : (q + 1) * 512],
                in_=xat_r[:, :, :, q * 512 : (q + 1) * 512],
            )

        acc_se = consts.tile([128, 32], f32)
        acc_mx = consts.tile([128, 32], f32)

        # ~16 dummy matmuls (~3.4us of PE work) bridge the initial DMA.
        for _ in range(16):
            pw = psumsB.tile([128, 512], f32, tag="psB")
            nc.tensor.matmul(pw, warm[:, 0:128], warm, start=True, stop=True)

        # Phase A: cosine vs class shard, fused exp-sum + row-max.
        for rt in range(32):
            src = s_xpt if rt < 16 else s_xat
            m0 = (rt % 16) * 128
            ps = psums.tile([128, CSH], f32, tag="psA")
            for c in range(2):
                nc.tensor.matmul(
                    ps[:, 0:512],
                    src[:, c, :, m0 : m0 + 128],
                    s_wnt[:, c, :, 0:512],
                    start=(c == 0),
                    stop=(c == 1),
                    perf_mode=DR,
                )
                nc.tensor.matmul(
                    ps[:, 512:CSH],
                    src[:, c, :, m0 : m0 + 128],
                    s_wnt[:, c, :, 512:CSH],
                    start=(c == 0),
                    stop=(c == 1),
                    perf_mode=DR,
                )
            e = scratch.tile([128, CSH], f32, tag="expA")
            nc.scalar.activation(
                e, ps, EXP, scale=SCALE, accum_out=acc_se[:, rt : rt + 1]
            )
            nc.vector.reduce_max(acc_mx[:, rt : rt + 1], ps, axis=AX)

        # Phase B: D = Xp @ Xa_shard.T; ship raw bf16 tiles, host does exp.
        # Copies run on the (mostly idle) Scalar engine so DVE keeps the
        # phase-A row-max pipeline; 4 row-tiles are staged per output DMA.
        dstage = consts.tile([128, 16, ASH], bf16)
        for rt in range(16):
            m0 = rt * 128
            ps = psumsB.tile([128, ASH], f32, tag="psB")
            for c in range(4):
                nc.tensor.matmul(
                    ps,
                    s_xpt[:, c // 2, c % 2, m0 : m0 + 128],
                    s_xash[:, c // 2, c % 2, :],
                    start=(c == 0),
                    stop=(c == 3),
                )
            nc.vector.tensor_copy(dstage[:, rt, :], ps)
            if rt % 4 == 3:
                nc.sync.dma_start(
                    out=o_d[rt - 3 : rt + 1].rearrange("r p n -> p r n"),
                    in_=dstage[:, rt - 3 : rt + 1, :],
                )

        nc.sync.dma_start(out=o_se, in_=acc_se)
        nc.sync.dma_start(out=o_mx, in_=acc_mx)

    nc.compile()
    return nc


def _numpy_fallback(x, weight, w, b, w2, w3, b2, b3, label):
    """Exact float64 implementation of the reference (general w2/b2 path)."""
    x = np.asarray(x, np.float64)
    weight = np.asarray(weight, np.float64)
    label = np.asarray(label).astype(np.int64)
    w, b, w2, w3, b2, b3 = (float(v) for v in (w, b, w2, w3, b2, b3))

    def l2n(v):
        return v / np.maximum(np.linalg.norm(v, axis=-1, keepdims=True), 1e-12)

    def ce(logits, labels):
        m = logits.max(-1, keepdims=True)
        lse = np.log(np.exp(logits - m).sum(-1)) + m[:, 0]
        tgt = logits[np.arange(len(labels)), labels]
        return np.mean(lse - tgt)

    bsz = x.shape[0]
    xf = x.reshape(-1, NOUT)
    lab2 = np.repeat(label, 2)
    xn = l2n(xf)
    wn = l2n(weight)
    cosine = xn @ wn.T
    sine = np.sqrt(np.clip(1.0 - cosine * cosine, 0.0, 1.0))
    phi = cosine * _COS_M - sine * _SIN_M
    phi = np.where(cosine - _TH > 0, phi, cosine - _MM)
    one_hot = np.zeros_like(cosine)
    one_hot[np.arange(2 * bsz), lab2] = 1.0
    output = (one_hot * phi + (1.0 - one_hot) * cosine) * SCALE
    nlossS = ce(output, lab2)
    prec1 = np.mean(output.argmax(-1) == lab2) * 100.0

    cosr = cosine.reshape(bsz, 2, NCLS)

    def snorm(xr0, xr1, cos0, cos1):
        # xr0/cos0 = positive slot, xr1/cos1 = anchor slot
        out_dot = l2n(xr0) @ l2n(xr1).T
        COHORT = 101

        def stats(c):
            top = -np.partition(-c, COHORT - 1, axis=-1)[:, :COHORT]
            return top.mean(-1), top.std(-1, ddof=1)

        mean1, std1 = stats(cos1)
        mean2, std2 = stats(cos0)
        od1 = (out_dot - _hsig(mean1 * w2 + w3)[None, :]) / _hsig(
            std1 * b2 + b3
        )[None, :]
        od2 = (out_dot - _hsig(mean2 * w2 + w3)[:, None]) / _hsig(
            std2 * b2 + b3
        )[:, None]
        csm = 0.5 * (od1 + od2) * w + b
        return ce(csm, np.arange(bsz))

    xr = xf.reshape(bsz, 2, NOUT)
    p1 = snorm(xr[:, 0], xr[:, 1], cosr[:, 0], cosr[:, 1])
    p2 = snorm(xr[:, 1], xr[:, 0], cosr[:, 1], cosr[:, 0])
    nlossP = 0.5 * (p1 + p2)
    return np.asarray([nlossS + nlossP, prec1], np.float32)


def kernel(x, weight, w, b, w2, w3, b2, b3, label):
    global last_results
    w_f, b_f, w2_f, w3_f, b2_f, b3_f = (
        float(np.asarray(v)) for v in (w, b, w2, w3, b2, b3)
    )
    if w2_f != 0.0 or b2_f != 0.0 or _hsig(b3_f) <= 0.0:
        return _numpy_fallback(x, weight, w, b, w2, w3, b2, b3, label)

    from concourse.bass_utils import run_bass_kernel_spmd

    x = np.asarray(x, np.float32)
    weight = np.asarray(weight, np.float32)
    label = np.asarray(label).astype(np.int64)

    # ---- host prep: normalize, quantize to bf16, transpose, shard ----
    xf = x.reshape(R, NOUT)
    xn = xf / np.maximum(np.linalg.norm(xf, axis=-1, keepdims=True), 1e-12)
    wn = weight / np.maximum(np.linalg.norm(weight, axis=-1, keepdims=True), 1e-12)
    xn16 = xn.astype(ml_dtypes.float8_e4m3)
    wn16 = wn.astype(ml_dtypes.float8_e4m3)

    XpT = np.ascontiguousarray(xn16[0::2].T)  # [512, 2048]
    XaT = np.ascontiguousarray(xn16[1::2].T)  # [512, 2048]
    WnT = np.zeros((NOUT, NCORES * CSH), ml_dtypes.float8_e4m3)
    WnT[:, :NCLS] = wn16.T

    in_maps = [
        {
            "xpt": XpT,
            "xat": XaT,
            "xash": np.ascontiguousarray(XaT[:, k * ASH : (k + 1) * ASH]),
            "wnt": np.ascontiguousarray(WnT[:, k * CSH : (k + 1) * CSH]),
        }
        for k in range(NCORES)
    ]

    m_ = _hsig(w3_f)
    s_ = _hsig(b3_f)
    alpha = w_f / s_

    if "prog" not in _cache:
        _cache["prog"] = _build_program()
    nc = _cache["prog"]

    res = run_bass_kernel_spmd(nc, in_maps, list(range(NCORES)))
    last_results = res

    # ---- host combine (float64) ----
    # Phase A partials: [128, 32] where row index = (rt % 16) * 128 + p,
    # rt < 16 -> positive rows (xf rows 0,2,4,...), rt >= 16 -> anchor rows.
    se = np.zeros((128, 32), np.float64)
    mx = np.full((128, 32), -np.inf)
    rowSE = np.zeros((B,), np.float64)
    cse = np.zeros((B,), np.float64)
    for k in range(NCORES):
        r = res.results[k]
        part = np.asarray(r["o_se"], np.float64)
        if k == NCORES - 1:
            part = part - float(NPAD)  # zero-padded classes contribute exp(0)=1
        se += part
        mx = np.maximum(mx, np.asarray(r["o_mx"], np.float64))
        # o_d[rt, p, j]: D for Xp row rt*128+p vs anchor k*ASH+j
        ed = np.exp(alpha * np.asarray(r["o_d"], np.float64))
        rowSE += ed.sum(axis=2).reshape(-1)
        cse[k * ASH : (k + 1) * ASH] = ed.sum(axis=(0, 1))

    # Map [128, 32] tiles back to row-major [4096] (interleaved pos/anchor).
    def tiles_to_rows(t):  # t: [128, 32] -> [4096] in xf row order
        pos = t[:, :16].T.reshape(-1)  # Xp index i -> xf row 2i
        anc = t[:, 16:].T.reshape(-1)
        out = np.empty(R, np.float64)
        out[0::2] = pos
        out[1::2] = anc
        return out

    sumexp = tiles_to_rows(se)
    M = tiles_to_rows(mx)

    # Target cosines / diag from the same bf16-quantized operands.
    xn16f = xn16.astype(np.float64)
    wn16f = wn16.astype(np.float64)
    lab2 = np.repeat(label, 2)
    c_t = np.einsum("ij,ij->i", xn16f, wn16f[lab2])
    d = np.einsum("ij,ij->i", xn16f[0::2], xn16f[1::2])

    sine = np.sqrt(np.clip(1.0 - c_t * c_t, 0.0, 1.0))
    phi = np.where(c_t - _TH > 0, c_t * _COS_M - sine * _SIN_M, c_t - _MM)
    lse = np.log(sumexp - np.exp(SCALE * c_t) + np.exp(SCALE * phi))
    nlossS = np.mean(lse - SCALE * phi)
    prec1 = 100.0 * np.mean(phi > M)

    p1 = np.mean(np.log(rowSE) - alpha * d)
    p2 = np.mean(np.log(cse) - alpha * d)
    nlossP = 0.5 * (p1 + p2)

    return np.asarray([nlossS + nlossP, prec1], np.float32)



# revision 4
# speedup vs baseline: 1.0401x; 1.0401x over previous
"""Trainium2 Bass kernel for nn_LossFunction_2740189135094 (AAM-softmax +
score-normalized angle-proto speaker loss).

Contract: kernel(**inputs) takes FULL unsharded inputs (as produced by the
reference setup_inputs) and returns the full output: a (2,) float32 array
[nlossS + nlossP, prec1].

Strategy (8 NeuronCores, class-sharded; tiny partial outputs merged on host):
  Each core owns 752 of the (padded-to-6016) 5994 classes and 256 of the
  2048 anchors. Per 128-row tile the PE computes, in one fused [128, 1008]
  PSUM tile, cosine vs the class shard (fp8 DoubleRow) and - for positive
  rows - the angle-proto similarity D vs the anchor shard. The row-wise
  sum(exp(30*cos)) is computed by BOTH the ACT engine (true exp + fused
  accumulate) and the DVE (Schraudolph bit-trick exp: affine f32->uint16,
  bitcast to bf16, 4x-mode accumulate), split per a static assignment so the
  two engines finish together. exp(alpha*D) row sums come from the same
  machinery; anchor-column sums come from a ones-vector matmul on the PE
  accumulated across row tiles in PSUM.

  There is no on-device row max: prec1 = mean(phi > max_{j!=label} cos_j)
  is decided on the host from the row-wise sum(exp): max >= log(sum/5993)/30.
  For this loss phi sits far below that bound; rows too close to the bound
  (never, in practice) fall back to an exact host check.

  Host does: l2-normalization, fp8-e4m3 quantization, layout packing (so
  every DMA is contiguous per partition - the input load is descriptor-bound
  otherwise), the label-gathered target cosines from the same fp8 operands,
  and the final logs/means in float64.

The top-k cohort statistics in the reference are multiplied by w2/b2; for
the actual inputs w2 == b2 == 0, so csm is an affine function of out_dot and
p2's matrix is exactly p1's transpose. If w2/b2 were nonzero we fall back to
an exact numpy implementation.
"""

import math
import sys

import numpy as np

for _p in ("/opt/trn_rl_repo", "/opt/pypackages"):
    if _p not in sys.path:
        sys.path.insert(0, _p)

import ml_dtypes  # noqa: E402

NOUT = 512
NCLS = 5994
B = 2048
R = 4096  # 2 * B rows
NCORES = 8
CSH = 752  # padded class shard: 8 * 752 = 6016 >= 5994
NPAD = NCORES * CSH - NCLS  # 22 zero-padded classes on the last core
ASH = B // NCORES  # 256 anchors per core
MARGIN = 0.2
SCALE = 30.0
PSW = CSH + ASH  # fused psum tile width: classes + anchor shard

_COS_M = math.cos(MARGIN)
_SIN_M = math.sin(MARGIN)
_TH = math.cos(math.pi - MARGIN)
_MM = math.sin(math.pi - MARGIN) * MARGIN

LOG2E = 1.4426950408889634
# Schraudolph bf16 exp: exp(s*c) ~= bitcast_bf16(uint16(c*(128*s*log2e) + BC)).
# BC is bias-free for sums: 128*(127 - log2(E_f[(1+f)/2^f])).
SCH_B = 128.0 * 127.0 - 128.0 * math.log2(1.0407419227)

# Engine assignment: 'A' = ACT (true exp, fused accum), 'D' = DVE
# (Schraudolph). Balanced so both engines finish together:
#   ACT class tile ~957ns, DVE class tile ~1160ns,
#   ACT D tile ~543ns, DVE D tile ~520ns.
N_ACT_CLS = 19
ASSIGN_CLS = [
    "A" if (i * N_ACT_CLS) // 32 != ((i + 1) * N_ACT_CLS) // 32 else "D"
    for i in range(32)
]
N_ACT_D = 4
ASSIGN_D = [
    "A" if (i * N_ACT_D) // 16 != ((i + 1) * N_ACT_D) // 16 else "D"
    for i in range(16)
]

_cache: dict = {}

# Results of the last device run (for the test harness to inspect timing).
last_results = None


def _hsig(v):
    return np.clip((v + 3.0) / 6.0, 0.0, 1.0)


def _sch_exp_np(c, s):
    """Replicate the device's Schraudolph exp in numpy (float64 out)."""
    a = np.float32(128.0 * s * LOG2E)
    t = np.asarray(c, np.float32) * a + np.float32(SCH_B)
    i = np.round(t).astype(np.uint16)
    return i.view(ml_dtypes.bfloat16).astype(np.float64)


def _build_program(alpha):
    import concourse.mybir as mybir
    import concourse.tile as tile
    from concourse import bacc
    from contextlib import ExitStack

    bf16 = mybir.dt.bfloat16
    u16 = mybir.dt.uint16
    f8 = mybir.dt.float8e4
    f32 = mybir.dt.float32
    DR = mybir.MatmulPerfMode.DoubleRow
    EXP = mybir.ActivationFunctionType.Exp
    MULT = mybir.AluOpType.mult
    ADD = mybir.AluOpType.add

    sch_a_cls = float(np.float32(128.0 * SCALE * LOG2E))
    sch_a_d = float(np.float32(128.0 * alpha * LOG2E))
    sch_b = float(np.float32(SCH_B))

    nc = bacc.Bacc(
        "TRN2", target_bir_lowering=False, debug=False, num_devices=NCORES
    )
    # Inputs are pre-packed on host so each DMA is one contiguous run per
    # partition (the load is descriptor-count-bound otherwise).
    # Layout [p][q][c][r][m]: element = operand[k, col] with k = c*256+r*128+p
    # (contraction index, DoubleRow-packed) and col = q*512+m.
    xpt = nc.dram_tensor("xpt", [128, 4, 2, 2, 512], f8, kind="ExternalInput").ap()
    xat = nc.dram_tensor("xat", [128, 4, 2, 2, 512], f8, kind="ExternalInput").ap()
    xash = nc.dram_tensor("xash", [128, 2, 2, ASH], f8, kind="ExternalInput").ap()
    wnt = nc.dram_tensor("wnt", [128, 2, 2, CSH], f8, kind="ExternalInput").ap()
    o_se_a = nc.dram_tensor("o_se_a", [128, 32], f32, kind="ExternalOutput").ap()
    o_se_d = nc.dram_tensor("o_se_d", [128, 32], f32, kind="ExternalOutput").ap()
    o_rs_a = nc.dram_tensor("o_rs_a", [128, 16], f32, kind="ExternalOutput").ap()
    o_rs_d = nc.dram_tensor("o_rs_d", [128, 16], f32, kind="ExternalOutput").ap()
    o_cs = nc.dram_tensor("o_cs", [1, ASH], f32, kind="ExternalOutput").ap()

    with tile.TileContext(nc) as tc, ExitStack() as ctx:
        consts = ctx.enter_context(tc.tile_pool(name="consts", bufs=1))
        psums = ctx.enter_context(tc.tile_pool(name="psums", bufs=3, space="PSUM"))
        psum_cs = ctx.enter_context(
            tc.tile_pool(name="psum_cs", bufs=1, space="PSUM")
        )
        dpool = ctx.enter_context(tc.tile_pool(name="dpool", bufs=2))

        # Constants initialized on the otherwise-idle GpSimd engine.
        warm = consts.tile([128, 512], bf16)
        nc.gpsimd.memset(warm, 0.0)
        ones = consts.tile([128, 1], bf16)
        nc.gpsimd.memset(ones, 1.0)

        # SBUF input tiles; DMA issue order == consumption order.
        s_wnt = consts.tile([128, 2, 2, CSH], f8)
        nc.sync.dma_start(out=s_wnt, in_=wnt)
        s_xpt = consts.tile([128, 4, 2, 2, 512], f8)
        nc.sync.dma_start(out=s_xpt[:, 0], in_=xpt[:, 0])
        s_xash = consts.tile([128, 2, 2, ASH], f8)
        nc.sync.dma_start(out=s_xash, in_=xash)
        for q in range(1, 4):
            nc.sync.dma_start(out=s_xpt[:, q], in_=xpt[:, q])
        s_xat = consts.tile([128, 4, 2, 2, 512], f8)
        for q in range(4):
            nc.sync.dma_start(out=s_xat[:, q], in_=xat[:, q])

        # Row-sum accumulators (each engine owns its own tiles; host merges).
        acc_se_a = consts.tile([128, 32], f32)
        acc_se_d = consts.tile([128, 32], f32)
        acc_rs_a = consts.tile([128, 16], f32)
        acc_rs_d = consts.tile([128, 16], f32)
        # ACT class-exp output is discarded; one shared scratch is enough
        # (ACT program order serializes its writers).
        act_scr = consts.tile([128, CSH], bf16)
        # DVE class Schraudolph bits; read only by the next DVE instruction.
        dve_scr = consts.tile([128, CSH], u16)
        cs_sb = consts.tile([1, ASH], f32)

        # exp(alpha*D) tiles, read by the PE ones-matmul (column sums).
        # cse accumulates over all 16 row tiles in one PSUM bank.
        cse = psum_cs.tile([1, ASH], f32)

        # PE warm-up: ramp the HAM clock while the first inputs stream in.
        for _ in range(6):
            pw = psums.tile([128, PSW], f32, tag="ps")
            nc.tensor.matmul(
                pw[:, 0:512], warm[:, 0:128], warm, start=True, stop=True
            )

        for rt in range(32):
            src = s_xpt if rt < 16 else s_xat
            q, m0 = (rt % 16) // 4, ((rt % 16) % 4) * 128
            ps = psums.tile([128, PSW], f32, tag="ps")
            for c in range(2):
                lhsT = src[:, q, c, :, m0 : m0 + 128]
                nc.tensor.matmul(
                    ps[:, 0:512],
                    lhsT,
                    s_wnt[:, c, :, 0:512],
                    start=(c == 0),
                    stop=(c == 1),
                    perf_mode=DR,
                )
                nc.tensor.matmul(
                    ps[:, 512:CSH],
                    lhsT,
                    s_wnt[:, c, :, 512:CSH],
                    start=(c == 0),
                    stop=(c == 1),
                    perf_mode=DR,
                )
                if rt < 16:
                    nc.tensor.matmul(
                        ps[:, CSH:PSW],
                        lhsT,
                        s_xash[:, c],
                        start=(c == 0),
                        stop=(c == 1),
                        perf_mode=DR,
                    )

            # sum_j exp(SCALE * cos) for this row tile.
            if ASSIGN_CLS[rt] == "A":
                nc.scalar.activation(
                    act_scr,
                    ps[:, 0:CSH],
                    EXP,
                    scale=SCALE,
                    accum_out=acc_se_a[:, rt : rt + 1],
                )
            else:
                nc.vector.tensor_scalar(
                    dve_scr, ps[:, 0:CSH], sch_a_cls, sch_b, MULT, ADD
                )
                eview = dve_scr.bitcast(bf16)
                nc.vector.tensor_scalar(
                    eview,
                    eview,
                    1.0,
                    None,
                    MULT,
                    ADD,
                    accum_out=acc_se_d[:, rt : rt + 1],
                )

            # exp(alpha * D): row sums via accum, bf16 values for the
            # column-sum ones-matmul.
            if rt < 16:
                if ASSIGN_D[rt] == "A":
                    dexp = dpool.tile([128, ASH], bf16, tag="dexp")
                    nc.scalar.activation(
                        dexp,
                        ps[:, CSH:PSW],
                        EXP,
                        scale=alpha,
                        accum_out=acc_rs_a[:, rt : rt + 1],
                    )
                    dmm = dexp
                else:
                    du16 = dpool.tile([128, ASH], u16, tag="du16")
                    nc.vector.tensor_scalar(
                        du16, ps[:, CSH:PSW], sch_a_d, sch_b, MULT, ADD
                    )
                    dmm = du16.bitcast(bf16)
                    nc.vector.tensor_scalar(
                        dmm,
                        dmm,
                        1.0,
                        None,
                        MULT,
                        ADD,
                        accum_out=acc_rs_d[:, rt : rt + 1],
                    )
                nc.tensor.matmul(
                    cse, ones, dmm, start=(rt == 0), stop=(rt == 15)
                )

        nc.vector.tensor_copy(cs_sb, cse)
        nc.sync.dma_start(out=o_se_a, in_=acc_se_a)
        nc.sync.dma_start(out=o_se_d, in_=acc_se_d)
        nc.sync.dma_start(out=o_rs_a, in_=acc_rs_a)
        nc.sync.dma_start(out=o_rs_d, in_=acc_rs_d)
        nc.sync.dma_start(out=o_cs, in_=cs_sb)

    nc.compile()
    return nc


def _numpy_fallback(x, weight, w, b, w2, w3, b2, b3, label):
    """Exact float64 implementation of the reference (general w2/b2 path)."""
    x = np.asarray(x, np.float64)
    weight = np.asarray(weight, np.float64)
    label = np.asarray(label).astype(np.int64)
    w, b, w2, w3, b2, b3 = (float(v) for v in (w, b, w2, w3, b2, b3))

    def l2n(v):
        return v / np.maximum(np.linalg.norm(v, axis=-1, keepdims=True), 1e-12)

    def ce(logits, labels):
        m = logits.max(-1, keepdims=True)
        lse = np.log(np.exp(logits - m).sum(-1)) + m[:, 0]
        tgt = logits[np.arange(len(labels)), labels]
        return np.mean(lse - tgt)

    bsz = x.shape[0]
    xf = x.reshape(-1, NOUT)
    lab2 = np.repeat(label, 2)
    xn = l2n(xf)
    wn = l2n(weight)
    cosine = xn @ wn.T
    sine = np.sqrt(np.clip(1.0 - cosine * cosine, 0.0, 1.0))
    phi = cosine * _COS_M - sine * _SIN_M
    phi = np.where(cosine - _TH > 0, phi, cosine - _MM)
    one_hot = np.zeros_like(cosine)
    one_hot[np.arange(2 * bsz), lab2] = 1.0
    output = (one_hot * phi + (1.0 - one_hot) * cosine) * SCALE
    nlossS = ce(output, lab2)
    prec1 = np.mean(output.argmax(-1) == lab2) * 100.0

    cosr = cosine.reshape(bsz, 2, NCLS)

    def snorm(xr0, xr1, cos0, cos1):
        # xr0/cos0 = positive slot, xr1/cos1 = anchor slot
        out_dot = l2n(xr0) @ l2n(xr1).T
        COHORT = 101

        def stats(c):
            top = -np.partition(-c, COHORT - 1, axis=-1)[:, :COHORT]
            return top.mean(-1), top.std(-1, ddof=1)

        mean1, std1 = stats(cos1)
        mean2, std2 = stats(cos0)
        od1 = (out_dot - _hsig(mean1 * w2 + w3)[None, :]) / _hsig(
            std1 * b2 + b3
        )[None, :]
        od2 = (out_dot - _hsig(mean2 * w2 + w3)[:, None]) / _hsig(
            std2 * b2 + b3
        )[:, None]
        csm = 0.5 * (od1 + od2) * w + b
        return ce(csm, np.arange(bsz))

    xr = xf.reshape(bsz, 2, NOUT)
    p1 = snorm(xr[:, 0], xr[:, 1], cosr[:, 0], cosr[:, 1])
    p2 = snorm(xr[:, 1], xr[:, 0], cosr[:, 1], cosr[:, 0])
    nlossP = 0.5 * (p1 + p2)
    return np.asarray([nlossS + nlossP, prec1], np.float32)


def _pack_dr(opT):
    """[512, N] fp8 operand -> [128, N/512, 2, 2, 512] DoubleRow DMA layout."""
    n = opT.shape[1]
    # [c, r, p, col] with k = c*256 + r*128 + p
    a = opT.reshape(2, 2, 128, n)
    # -> [p, q, c, r, m]
    a = a.transpose(2, 0, 1, 3).reshape(128, 2, 2, n // 512, 512)
    return np.ascontiguousarray(a.transpose(0, 3, 1, 2, 4))


def kernel(x, weight, w, b, w2, w3, b2, b3, label):
    global last_results
    w_f, b_f, w2_f, w3_f, b2_f, b3_f = (
        float(np.asarray(v)) for v in (w, b, w2, w3, b2, b3)
    )
    if w2_f != 0.0 or b2_f != 0.0 or _hsig(b3_f) <= 0.0:
        return _numpy_fallback(x, weight, w, b, w2, w3, b2, b3, label)

    from concourse.bass_utils import run_bass_kernel_spmd

    x = np.asarray(x, np.float32)
    weight = np.asarray(weight, np.float32)
    label = np.asarray(label).astype(np.int64)
    alpha = w_f / _hsig(b3_f)

    # ---- host prep: normalize, quantize to fp8, pack DMA layouts ----
    xf = x.reshape(R, NOUT)
    xn = xf / np.maximum(np.linalg.norm(xf, axis=-1, keepdims=True), 1e-12)
    wn = weight / np.maximum(np.linalg.norm(weight, axis=-1, keepdims=True), 1e-12)
    xn8 = xn.astype(ml_dtypes.float8_e4m3)
    wn8 = wn.astype(ml_dtypes.float8_e4m3)

    XpT = np.ascontiguousarray(xn8[0::2].T)  # [512, 2048]
    XaT = np.ascontiguousarray(xn8[1::2].T)  # [512, 2048]
    WnT = np.zeros((NOUT, NCORES * CSH), ml_dtypes.float8_e4m3)
    WnT[:, :NCLS] = wn8.T

    xpt_p = _pack_dr(XpT)
    xat_p = _pack_dr(XaT)
    in_maps = []
    for k in range(NCORES):
        wsh = np.ascontiguousarray(WnT[:, k * CSH : (k + 1) * CSH])
        ash = np.ascontiguousarray(XaT[:, k * ASH : (k + 1) * ASH])
        in_maps.append(
            {
                "xpt": xpt_p,
                "xat": xat_p,
                # [p][c][r][col] packing for the 752/256-wide operands
                "wnt": np.ascontiguousarray(
                    wsh.reshape(2, 2, 128, CSH).transpose(2, 0, 1, 3)
                ),
                "xash": np.ascontiguousarray(
                    ash.reshape(2, 2, 128, ASH).transpose(2, 0, 1, 3)
                ),
            }
        )

    key = ("prog", alpha)
    if key not in _cache:
        _cache[key] = _build_program(alpha)
    nc = _cache[key]

    res = run_bass_kernel_spmd(nc, in_maps, list(range(NCORES)))
    last_results = res

    # ---- host combine (float64) ----
    # Row tiling: rt < 16 -> positive rows (xf rows 0,2,...), rt >= 16 ->
    # anchor rows; row = (rt % 16) * 128 + p.
    pad_a = 1.0
    pad_d = float(_sch_exp_np(np.zeros(1), SCALE)[0])
    se = np.zeros((128, 32), np.float64)
    rowSE = np.zeros((B,), np.float64)
    cse = np.zeros((B,), np.float64)
    for k in range(NCORES):
        r = res.results[k]
        se_a = np.asarray(r["o_se_a"], np.float64)
        se_d = np.asarray(r["o_se_d"], np.float64)
        for rt in range(32):
            col = se_a[:, rt] if ASSIGN_CLS[rt] == "A" else se_d[:, rt]
            if k == NCORES - 1:
                col = col - NPAD * (pad_a if ASSIGN_CLS[rt] == "A" else pad_d)
            se[:, rt] += col
        rs_a = np.asarray(r["o_rs_a"], np.float64)
        rs_d = np.asarray(r["o_rs_d"], np.float64)
        for rt in range(16):
            rowSE[rt * 128 : (rt + 1) * 128] += (
                rs_a[:, rt] if ASSIGN_D[rt] == "A" else rs_d[:, rt]
            )
        cse[k * ASH : (k + 1) * ASH] = np.asarray(r["o_cs"], np.float64)[0]

    def tiles_to_rows(t):  # t: [128, 32] -> [4096] in xf row order
        pos = t[:, :16].T.reshape(-1)
        anc = t[:, 16:].T.reshape(-1)
        out = np.empty(R, np.float64)
        out[0::2] = pos
        out[1::2] = anc
        return out

    sumexp = tiles_to_rows(se)

    # Target cosines / diag from the same fp8-quantized operands.
    xn8f = xn8.astype(np.float64)
    wn8f = wn8.astype(np.float64)
    lab2 = np.repeat(label, 2)
    c_t = np.einsum("ij,ij->i", xn8f, wn8f[lab2])
    d = np.einsum("ij,ij->i", xn8f[0::2], xn8f[1::2])

    # Device-replicated target term (engine of the row's tile).
    eng = np.empty(R, dtype="U1")
    for rt in range(32):
        base = 0 if rt < 16 else 1
        rows = 2 * ((rt % 16) * 128 + np.arange(128)) + base
        eng[rows] = ASSIGN_CLS[rt]
    t_dev = np.where(
        eng == "A", np.exp(SCALE * c_t), _sch_exp_np(c_t, SCALE)
    )

    sine = np.sqrt(np.clip(1.0 - c_t * c_t, 0.0, 1.0))
    phi = np.where(c_t - _TH > 0, c_t * _COS_M - sine * _SIN_M, c_t - _MM)
    se_no_t = np.maximum(sumexp - t_dev, 1e-300)
    lse = np.log(se_no_t + np.exp(SCALE * phi))
    nlossS = np.mean(lse - SCALE * phi)

    # prec1: argmax==label iff phi > max_{j!=label} cos_j. From the exp-sum,
    # max_{j!=label} >= log(se_no_t/5993)/SCALE; rows above that bound
    # (minus a safety margin for the ~3% Schraudolph error) get an exact check.
    lb = (np.log(se_no_t) - math.log(NCLS - 1)) / SCALE
    amb = phi > lb - 0.004
    prec_bits = np.zeros(R, bool)
    if amb.any():
        idx = np.where(amb)[0]
        cosf = xn8f[idx] @ wn8f.T
        cosf[np.arange(len(idx)), lab2[idx]] = -np.inf
        prec_bits[idx] = phi[idx] > cosf.max(axis=1)
    prec1 = 100.0 * prec_bits.mean()

    p1 = np.mean(np.log(rowSE) - alpha * d)
    p2 = np.mean(np.log(cse) - alpha * d)
    nlossP = 0.5 * (p1 + p2)

    return np.asarray([nlossS + nlossP, prec1], np.float32)
